# revision 26
# baseline (speedup 1.0000x reference)
"""GAT-EFA Trainium2 kernel (8 NeuronCores, SPMD).

Host side folds the tiled-input linears and derives the exact piecewise-linear
(PWL) representation of the per-edge attention score cp_h(e) for all 12 heads:
for adj==1 the edge_feats -> wf -> af chain is a scalar PWL function of the
edge value.  The adjacency mask folds in as an extra relu(-e') basis row
(e' = e - 1e6 where adj==0) with a large negative Gamma coefficient, so masked
logits underflow exp() to exactly 0.

Device (per core): 50 nodes x 2 batches = 100 attention rows, all 12 heads.
Rows are ordered (2*n_local + b).  Attention supertiles [128, 400] use
partitions 32k+h (4 rows x 12 heads).  One AllGather moves layer-0 outputs
between the GAT layers.  All precision-critical fp32 matmuls keep K <= 64.
"""
import sys
sys.path.insert(0, '/opt/trn_rl_repo')
import numpy as np
import concourse.bass as bass
import concourse.bacc as bacc
import concourse.tile as tile
import concourse.mybir as mybir
from concourse import bass_utils
from concourse.masks import make_identity

F32 = mybir.dt.float32
AF = mybir.ActivationFunctionType
AL = mybir.AluOpType

N_CORES = 8
B, N, NFEAT, NHID = 2, 400, 128, 64
NH0, NH1, NHEADS = 8, 4, 12
FG = 10
NPC = N // N_CORES            # 50 nodes per core
RPC = NPC * B                 # 100 rows per core
NST = RPC // 4                # 25 phase-A supertiles
MASK_OFF = 1.0e6
MASK_COEF = -2000.0

_relu = lambda x: np.maximum(x, 0.0)
_leaky = lambda x: np.where(x > 0, x, 0.2 * x)


# =========================================================== host-side folding
def _fold_rep(w, d):
    return w.reshape(w.shape[0], FG, d).sum(axis=1)


def _derive(params):
    d = {}
    p64 = lambda a: np.asarray(a, np.float64)

    Wn = np.zeros((128, 6))
    bn = np.zeros(128)
    for name, dim, ro, co in [('node_w1', 2, 0, 0), ('node_w2', 1, 32, 2),
                              ('node_w3', 2, 64, 3), ('node_w4', 1, 96, 5)]:
        Wn[ro:ro + 32, co:co + dim] = _fold_rep(p64(params[name]['w']), dim)
        bn[ro:ro + 32] = p64(params[name]['b'])
    d['Wn_T'] = Wn.T.astype(np.float32)
    d['bn'] = bn.astype(np.float32)
    d['Wemb_T'] = p64(params['node_emb']['w']).T.astype(np.float32)
    d['bemb'] = np.asarray(params['node_emb']['b'], np.float32)

    s1 = p64(params['edge_w1']['w']).sum(axis=1)
    b1 = p64(params['edge_w1']['b'])
    s2 = p64(params['edge_w2']['w']).sum(axis=1)
    b2 = p64(params['edge_w2']['b'])
    E = p64(params['edge_emb']['w'])
    q = E[:, 32:] @ _relu(s2 + b2) + p64(params['edge_emb']['b'])
    E1 = E[:, :32]
    heads = list(params['layer0']) + list(params['layer1'])

    def cp_all(e):
        z1 = _relu(np.multiply.outer(e, s1) + b1)
        ef = _relu(z1 @ E1.T + q)
        outs = []
        for p in heads:
            f = _leaky(ef @ p64(p['wf']['w']).T + p64(p['wf']['b']) + p64(p['bf']))
            outs.append(_leaky(f @ p64(p['af']['w'][0]) + p64(p['af']['b'][0])))
        return np.stack(outs, axis=-1)

    kinks = set()
    for k in range(32):
        if s1[k] != 0.0:
            t = -b1[k] / s1[k]
            if 0.0 < t < 1.0:
                kinks.add(float(t))

    def crossings(fn, nunits, T):
        pts = np.array([0.0] + sorted(T) + [1.0])
        new = set()
        for i in range(len(pts) - 1):
            a, b = pts[i], pts[i + 1]
            if b - a < 1e-12:
                continue
            fa = fn(np.array([a + 1e-12]))[0]
            fb = fn(np.array([b - 1e-12]))[0]
            for u in range(nunits):
                if fa[u] * fb[u] < 0:
                    t = a + (b - a) * (-fa[u]) / (fb[u] - fa[u])
                    for _ in range(3):
                        f0 = fn(np.array([t]))[0, u]
                        f1 = fn(np.array([t + 1e-9]))[0, u]
                        g = (f1 - f0) / 1e-9
                        if g != 0.0:
                            t = t - f0 / g
                        t = min(max(t, a), b)
                    if 0.0 < t < 1.0:
                        new.add(float(t))
        return new

    def ef_pre(e):
        z1 = _relu(np.multiply.outer(e, s1) + b1)
        return z1 @ E1.T + q

    kinks |= crossings(ef_pre, 64, kinks)

    def f_pre(e):
        z1 = _relu(np.multiply.outer(e, s1) + b1)
        ef = _relu(z1 @ E1.T + q)
        return np.concatenate(
            [ef @ p64(p['wf']['w']).T + p64(p['wf']['b']) + p64(p['bf'])
             for p in heads], axis=-1)

    kinks |= crossings(f_pre, 64 * NHEADS, kinks)

    def cp_pre(e):
        z1 = _relu(np.multiply.outer(e, s1) + b1)
        ef = _relu(z1 @ E1.T + q)
        return np.stack(
            [(_leaky(ef @ p64(p['wf']['w']).T + p64(p['wf']['b']) + p64(p['bf']))
              @ p64(p['af']['w'][0]) + p64(p['af']['b'][0])) for p in heads],
            axis=-1)

    kinks |= crossings(cp_pre, NHEADS, kinks)
    T = np.array(sorted(kinks))
    NK = len(T)
    assert NK <= 122, f"too many kinks: {NK}"

    pts = np.concatenate([[0.0], T, [1.0]])
    mids = (pts[:-1] + pts[1:]) / 2
    eps = 1e-7
    slopes = (cp_all(mids + eps) - cp_all(mids - eps)) / (2 * eps)
    c0 = cp_all(np.array([0.0]))[0]

    KB = NK + 3
    Gm = np.zeros((KB, NHEADS))
    Gm[0] = c0
    Gm[1] = slopes[0]
    Gm[2:2 + NK] = slopes[1:] - slopes[:-1]
    Gm[KB - 1] = MASK_COEF
    d['Gamma'] = Gm.astype(np.float32)
    sc = np.ones(KB); sc[0] = 0.0; sc[KB - 1] = -1.0
    bi = np.zeros(KB); bi[0] = 1.0; bi[2:2 + NK] = -T
    d['bas_scale'] = sc.astype(np.float32)
    d['bas_bias'] = bi.astype(np.float32)
    d['KB'] = KB

    def head_cols(p, in_d):
        rhs = np.zeros((in_d, 66))
        rhs[:, 0:64] = p64(p['wh']['w']).T
        rhs[:, 64] = p64(p['wh1']['w']).T @ p64(p['ah']['w'][0])
        rhs[:, 65] = p64(p['wh2']['w']).T @ p64(p['ah']['w'][0])
        bias = np.zeros(66)
        bias[0:64] = p64(p['wh']['b'])
        bias[64] = (p64(p['wh1']['b']) @ p64(p['ah']['w'][0])
                    + p64(p['wh2']['b']) @ p64(p['ah']['w'][0])
                    + p64(p['ah']['b'][0]))
        return rhs, bias

    r0 = [head_cols(p, NFEAT) for p in params['layer0']]
    d['rhs0'] = np.concatenate([r[0] for r in r0], axis=1).astype(np.float32)
    d['bias0'] = np.concatenate([r[1] for r in r0]).astype(np.float32)
    r1 = [head_cols(p, NHID * NH0) for p in params['layer1']]
    d['rhs1'] = np.concatenate([r[0] for r in r1], axis=1).astype(np.float32)
    d['bias1'] = np.concatenate([r[1] for r in r1]).astype(np.float32)

    # v1 columns + K-bias row for the per-core a1 matmuls
    v1k0 = np.zeros((NFEAT + 1, NH0), np.float32)
    for h, (rhs, bias) in enumerate(r0):
        v1k0[0:NFEAT, h] = rhs[:, 64]
        v1k0[NFEAT, h] = bias[64]
    d['v1k0'] = v1k0
    v1k1 = np.zeros((512 + 1, NH1), np.float32)
    for h, (rhs, bias) in enumerate(r1):
        v1k1[0:512, h] = rhs[:, 64]
        v1k1[512, h] = bias[64]
    d['v1k1'] = v1k1

    d['W1_T'] = p64(params['out1']['w']).T.astype(np.float32)
    d['b1'] = np.asarray(params['out1']['b'], np.float32)
    d['W2_T'] = p64(params['out2']['w']).T.astype(np.float32)
    d['b2'] = float(np.asarray(params['out2']['b']).ravel()[0])
    return d


# ============================================================== device program
def _sap(t, off, dims):
    """Raw AP into a tile/dram tensor at flat element offset `off`."""
    return bass.AP(tensor=t.tensor, offset=t.offset + off, ap=dims)


def build_program(KB, stop_after=99):
    nc = bacc.Bacc("TRN2", target_bir_lowering=False, debug=False,
                   num_devices=N_CORES)

    def din(name, shape):
        return nc.dram_tensor(name, shape, F32, kind="ExternalInput").ap()

    eadj_d = din("eadj", [RPC, 2 * N])
    xown_d = din("xown", [6, RPC])
    xall_d = din("xall", [6, B * N])
    nconst_d = din("nconst", [6, NFEAT])
    nbias_d = din("nbias", [NFEAT, 2])
    wemb_d = din("wemb", [NFEAT, NFEAT])
    basv_d = din("basv", [KB, 2])
    gam_d = din("gam", [KB, NHEADS])
    rhs0_d = din("rhs0", [NFEAT, 528])  # loaded as [64,2,528]
    brep0_d = din("brep0", [1, 528])
    v1k0_d = din("v1k0", [NFEAT + 1, NH0])
    rhs1_d = din("rhs1", [512, 264])
    brep1_d = din("brep1", [1, 264])
    v1k1_d = din("v1k1", [512 + 1, NH1])
    wout_d = din("wout", [NHID, 33])
    bout_d = din("bout", [32, 2])
    out_d = nc.dram_tensor("out", [1, RPC], F32, kind="ExternalOutput").ap()

    ep_h = nc.dram_tensor("ep_h", [RPC, N], F32).ap()
    a1h0_h = nc.dram_tensor("a1h0_h", [RPC, NH0], F32).ap()
    a2h0_h = nc.dram_tensor("a2h0_h", [B, NH0, N], F32).ap()
    a1h1_h = nc.dram_tensor("a1h1_h", [RPC, NH1], F32).ap()
    a2h1_h = nc.dram_tensor("a2h1_h", [B, NH1, N], F32).ap()
    cp1_h = nc.dram_tensor("cp1_h", [RPC, NH1, N], F32).ap()
    rs_h = nc.dram_tensor("rs_h", [128, NST], F32).ap()
    rs1_h = nc.dram_tensor("rs1_h", [128, 4], F32).ap()
    agin_h = nc.dram_tensor("agin_h", [NH0 * NHID, RPC], F32).ap()
    dbg1_h = nc.dram_tensor("dbg1_h", [NH1 * B * NHID, NPC], F32).ap()
    dbgr_h = nc.dram_tensor("dbgr_h", [NH1 * B * NHID, NPC], F32).ap()
    agout_h = nc.dram_tensor("agout_h", [N_CORES, NH0 * NHID, RPC], F32,
                             addr_space="Shared").ap()

    node_chunks = [(b * N + s, min(64, N - s), b)
                   for b in range(B) for s in range(0, N, 64)]
    NCHB = len(node_chunks) // B          # 7 chunks per batch

    with tile.TileContext(nc) as tc, \
         tc.tile_pool(name="con", bufs=1) as con, \
         tc.tile_pool(name="big", bufs=1) as big, \
         tc.tile_pool(name="wrk", bufs=2) as wrk, \
         tc.tile_pool(name="bas", bufs=2) as bas, \
         tc.tile_pool(name="ps_a", bufs=2, space="PSUM") as ps_a, \
         tc.tile_pool(name="ps_t", bufs=2, space="PSUM") as ps_t, \
         tc.tile_pool(name="ps_m", bufs=2, space="PSUM") as ps_m:

        # ---------------- constants
        ident = con.tile([128, 128], F32)
        make_identity(nc, ident[:])
        basv = con.tile([KB, 2], F32)
        nc.sync.dma_start(basv[:], basv_d)
        gam = con.tile([KB, NHEADS], F32)
        nc.sync.dma_start(gam[:], gam_d)
        nconst = con.tile([6, NFEAT], F32)
        nc.sync.dma_start(nconst[:], nconst_d)
        nbias = con.tile([NFEAT, 2], F32)
        nc.sync.dma_start(nbias[:], nbias_d)
        wemb = con.tile([64, 2, NFEAT], F32)
        nc.sync.dma_start(wemb[:], wemb_d.rearrange("(c p) n -> p c n", p=64))
        rhs0 = con.tile([64, 2, 528], F32)
        nc.sync.dma_start(rhs0[:], rhs0_d.rearrange("(c p) n -> p c n", p=64))
        brep0 = con.tile([128, 528], F32)
        nc.sync.dma_start(brep0[:], brep0_d.partition_broadcast(128)[:, 0, :])
        v1k0 = con.tile([64, 2, NH0], F32)
        nc.sync.dma_start(v1k0[:],
                          v1k0_d[0:NFEAT, :].rearrange("(c p) n -> p c n", p=64))
        v1k0K = con.tile([1, NH0], F32)
        nc.sync.dma_start(v1k0K[:], v1k0_d[NFEAT:NFEAT + 1, :])
        rhs1 = con.tile([64, 8, 264], F32)
        nc.sync.dma_start(rhs1[:], rhs1_d.rearrange("(c p) n -> p c n", p=64))
        brep1 = con.tile([128, 264], F32)
        nc.sync.dma_start(brep1[:], brep1_d.partition_broadcast(128)[:, 0, :])
        v1k1 = con.tile([64, 8, NH1], F32)
        nc.sync.dma_start(v1k1[:], v1k1_d[0:512, :].rearrange("(c p) n -> p c n", p=64))
        v1kK = con.tile([1, NH1], F32)
        nc.sync.dma_start(v1kK[:], v1k1_d[512:513, :])
        wout = con.tile([NHID, 33], F32)
        nc.sync.dma_start(wout[:], wout_d)
        bout = con.tile([32, 2], F32)
        nc.sync.dma_start(bout[:], bout_d)
        ones1 = con.tile([1, RPC], F32)
        nc.vector.memset(ones1[:], 1.0)
        ones64 = con.tile([64, 64], F32)
        nc.vector.memset(ones64[:], 1.0)
        epsc = con.tile([64, 1], F32)
        nc.vector.memset(epsc[:], 1e-5)

        # ---------------- e' = e + 1e6*adj - 1e6 -> HBM
        eadj = big.tile([RPC, 2 * N], F32)
        nc.sync.dma_start(eadj[:], eadj_d)
        ep = big.tile([RPC, N], F32)
        nc.vector.tensor_scalar(out=ep[:], in0=eadj[:, N:2 * N],
                                scalar1=MASK_OFF, scalar2=-MASK_OFF,
                                op0=AL.mult, op1=AL.add)
        nc.vector.tensor_tensor(out=ep[:], in0=ep[:], in1=eadj[:, 0:N],
                                op=AL.add)
        nc.sync.dma_start(ep_h, ep[:])

        # ---------------- node pipeline (global + own)
        def node_feats(src_ap, width, tag):
            xa = big.tile([6, width], F32, tag=f"xa{tag}")
            nc.sync.dma_start(xa[:], src_ap)
            xf = big.tile([NFEAT, width], F32, tag=f"xf{tag}")
            xf1 = big.tile([64, width], F32, tag=f"xf1{tag}")
            nf = big.tile([NFEAT, width], F32, tag=f"nf{tag}")
            nf1 = big.tile([64, width], F32, tag=f"nf1{tag}")
            for s in range(0, width, 400):
                w = min(400, width - s)
                pn = ps_m.tile([NFEAT, 400], F32, tag="ps_m")
                nc.tensor.matmul(pn[:, 0:w], nconst[:], xa[:, s:s + w],
                                 start=True, stop=True)
                nc.scalar.activation(xf[:, s:s + w], pn[:, 0:w], AF.Relu,
                                     bias=nbias[:, 0:1])
            nc.sync.dma_start(xf1[:], xf[64:128, :])
            for s in range(0, width, 400):
                w = min(400, width - s)
                pe = ps_m.tile([NFEAT, 400], F32, tag="ps_m")
                nc.tensor.matmul(pe[:, 0:w], wemb[:, 0, :],
                                 xf[0:64, s:s + w], start=True, stop=False)
                nc.tensor.matmul(pe[:, 0:w], wemb[:, 1, :],
                                 xf1[:, s:s + w], start=False, stop=True)
                nc.scalar.activation(nf[:, s:s + w], pe[:, 0:w], AF.Relu,
                                     bias=nbias[:, 1:2])
            nc.sync.dma_start(nf1[:], nf[64:128, :])
            return nf, nf1

        nfT, nfT1 = node_feats(xall_d, B * N, "g")
        nfo, nfo1 = node_feats(xown_d, RPC, "o")

        # layer-0 a1 for own rows -> a1h0_h [RPC, NH0]
        pa1 = ps_m.tile([RPC, NH0], F32, tag="ps_m")
        nc.tensor.matmul(pa1[:], nfo[0:64, :], v1k0[:, 0, :],
                         start=True, stop=False)
        nc.tensor.matmul(pa1[:], nfo1[:], v1k0[:, 1, :],
                         start=False, stop=False)
        nc.tensor.matmul(pa1[:], ones1[:], v1k0K[:],
                         start=False, stop=True)
        a1o = wrk.tile([RPC, NH0], F32, tag="a1o")
        nc.vector.tensor_copy(a1o[:], pa1[:])
        nc.sync.dma_start(a1h0_h, a1o[:])

        # ---------------- layer-0 A-matmuls [h|a1|a2] per head, all nodes
        a0out = []
        for ichk, (g0, sz, b) in enumerate(node_chunks):
            t = big.tile([64, 528], F32, tag=f"a0out{ichk}", name=f"a0out{ichk}")
            for half in range(2):
                cols = slice(264 * half, 264 * (half + 1))
                p = ps_m.tile([64, 264], F32, tag="ps_m")
                nc.tensor.matmul(p[0:sz, :], nfT[0:64, g0:g0 + sz],
                                 rhs0[:, 0, cols], start=True, stop=False)
                nc.tensor.matmul(p[0:sz, :], nfT1[:, g0:g0 + sz],
                                 rhs0[:, 1, cols], start=False, stop=True)
                nc.vector.tensor_tensor(out=t[0:sz, cols], in0=p[0:sz, :],
                                        in1=brep0[0:sz, cols], op=AL.add)
            a0out.append((t, g0, sz, b))

        # ---------------- a2 extraction (exact transposes)
        def extract_a2(aout, nh, width, a2_hbm, tag):
            for b in range(B):
                a2T = wrk.tile([nh, N], F32, tag=f"a2T{tag}")
                for ci in range(NCHB):
                    (t, g0, sz, _b) = aout[b * NCHB + ci]
                    pt = ps_t.tile([nh, 64], F32, tag="ps_t")
                    lhsT = _sap(t, 65, [[width, sz], [66, nh]])
                    nc.tensor.matmul(pt[:, 0:sz], lhsT, ident[0:sz, 0:sz],
                                     start=True, stop=True)
                    nc.vector.tensor_copy(a2T[:, 64 * ci:64 * ci + sz],
                                          pt[:, 0:sz])
                nc.sync.dma_start(a2_hbm[b], a2T[:])

        extract_a2(a0out, NH0, 528, a2h0_h, "0")
        if stop_after <= 1:
            nc.compile()
            return nc

        a2rep0 = big.tile([128, N], F32)
        nc.vector.memset(a2rep0[:], 0.0)
        for k in range(4):
            nc.sync.dma_start(a2rep0[32 * k:32 * k + NH0, :], a2h0_h[k % 2])

        # ---------------- phase A attention (12 heads)
        uT = [big.tile([64, 48 * NST], F32, tag=f"uT{c}", name=f"uT{c}") for c in range(7)]
        rs_all = big.tile([128, NST], F32)
        a1c_all = big.tile([128, NST], F32)
        nc.vector.memset(a1c_all[:], 0.0)
        for k in range(4):
            nc.sync.dma_start(
                a1c_all[32 * k:32 * k + NH0, :],
                _sap(a1h0_h, k * NH0, [[1, NH0], [4 * NH0, NST]]))

        for t in range(NST):
            ebc = bas.tile([KB, 4, N], F32, tag="ebc")
            nc.sync.dma_start(
                ebc[:], _sap(ep_h, t * 4 * N, [[0, KB], [N, 4], [1, N]]))
            pcp = ps_a.tile([128, N], F32, tag="pcp")
            nc.vector.memset(pcp[:], 0.0)
            for k in range(4):
                bt = bas.tile([KB, N], F32, tag="bt")
                nc.scalar.activation(bt[:], ebc[:, k, :], AF.Relu,
                                     bias=basv[:, 1:2], scale=basv[:, 0:1])
                nc.tensor.matmul(pcp[32 * k:32 * k + NHEADS, :], gam[:], bt[:],
                                 start=True, stop=True,
                                 tile_position=(0, 32 * k))
            cps = wrk.tile([128, N], F32, tag="cps")
            nc.vector.tensor_copy(cps[:], pcp[:])
            for k in range(4):
                nc.gpsimd.dma_start(cp1_h[4 * t + k],
                                    cps[32 * k + NH0:32 * k + NHEADS, :])
            pair = wrk.tile([128, N], F32, tag="pair")
            nc.vector.tensor_scalar_add(pair[:], a2rep0[:], a1c_all[:, t:t + 1])
            lk = wrk.tile([128, N], F32, tag="lk")
            nc.vector.tensor_scalar_mul(lk[:], pair[:], 0.2)
            nc.vector.tensor_tensor(out=lk[:], in0=lk[:], in1=pair[:], op=AL.max)
            cc = wrk.tile([128, N], F32, tag="cc")
            nc.vector.tensor_tensor(out=cc[:], in0=lk[:], in1=cps[:], op=AL.add)
            u = wrk.tile([128, N], F32, tag="u")
            nc.scalar.activation(u[:], cc[:], AF.Exp, accum_out=rs_all[:, t:t + 1])
            for c in range(7):
                w = min(64, N - 64 * c)
                ptr = ps_t.tile([64, 128], F32, tag="ps_t")
                nc.tensor.transpose(ptr[0:w, :], u[:, 64 * c:64 * c + w],
                                    ident[:])
                nc.vector.tensor_copy(
                    uT[c][0:w, 48 * t:48 * (t + 1)],
                    _sap(ptr, 0, [[128, w], [32, 4], [1, NHEADS]]))

        if stop_after <= 2:
            nc.compile()
            return nc
        rcp = wrk.tile([128, NST], F32, tag="rcp")
        nc.vector.memset(rcp[:], 1.0)
        for k in range(4):
            nc.vector.reciprocal(rcp[32 * k:32 * k + NHEADS, :],
                                 rs_all[32 * k:32 * k + NHEADS, :])
        nc.sync.dma_start(rs_h, rcp[:])

        # ---------------- aggregation + normalization helper
        def agg_norm(aout, uTl, nh, width, uTw, rhs_col, srep_dma, leaky, tag):
            raw = {}
            for h in range(nh):
                for b in range(B):
                    pg = ps_m.tile([NHID, NPC], F32, tag="ps_m")
                    pieces = [(ci, min(64, N - 64 * ci)) for ci in range(7)]
                    for idx, (ci, w) in enumerate(pieces):
                        (t, g0, sz, _b) = aout[b * NCHB + ci]
                        lhsT = _sap(t, 66 * h, [[width, w], [1, 64]])
                        rhs = _sap(uTl[ci], rhs_col(b, h),
                                   [[uTw, w]] + rhs_col(b, h, dims=True))
                        nc.tensor.matmul(pg[:], lhsT, rhs, start=(idx == 0),
                                         stop=(idx == len(pieces) - 1))
                    rw = wrk.tile([NHID, NPC], F32, tag=f"raw{tag}{h}{b}")
                    srep = wrk.tile([NHID, NPC], F32, tag=f"srep{tag}")
                    srep_dma(srep, b, h)
                    nc.vector.tensor_tensor(out=rw[:], in0=pg[:], in1=srep[:],
                                            op=AL.mult)
                    if tag == "b":
                        nc.gpsimd.dma_start(
                            _sap(dbgr_h, (h * B + b) * NHID * NPC,
                                 [[NPC, NHID], [1, NPC]]), rw[:])
                    raw[(h, b)] = rw
            out = {}
            for h in range(nh):
                pcs = ps_m.tile([64, NPC], F32, tag="ps_m")
                sq0 = wrk.tile([NHID, NPC], F32, tag=f"sqa{tag}")
                sq1 = wrk.tile([NHID, NPC], F32, tag=f"sqb{tag}")
                nc.vector.tensor_tensor(out=sq0[:], in0=raw[(h, 0)][:],
                                        in1=raw[(h, 0)][:], op=AL.mult)
                nc.vector.tensor_tensor(out=sq1[:], in0=raw[(h, 1)][:],
                                        in1=raw[(h, 1)][:], op=AL.mult)
                nc.tensor.matmul(pcs[:], ones64[:], raw[(h, 0)][:],
                                 start=True, stop=False)
                nc.tensor.matmul(pcs[:], ones64[:], raw[(h, 1)][:],
                                 start=False, stop=True)
                mean = wrk.tile([64, NPC], F32, tag=f"mean{tag}")
                nc.vector.tensor_scalar_mul(mean[:], pcs[:], 1.0 / 128.0)
                pss = ps_m.tile([64, NPC], F32, tag="ps_m")
                nc.tensor.matmul(pss[:], ones64[:], sq0[:], start=True, stop=False)
                nc.tensor.matmul(pss[:], ones64[:], sq1[:], start=False, stop=True)
                var = wrk.tile([64, NPC], F32, tag=f"var{tag}")
                nc.vector.tensor_scalar_mul(var[:], pss[:], 1.0 / 128.0)
                msq = wrk.tile([64, NPC], F32, tag=f"msq{tag}")
                nc.vector.tensor_tensor(out=msq[:], in0=mean[:], in1=mean[:],
                                        op=AL.mult)
                nc.vector.tensor_tensor(out=var[:], in0=var[:], in1=msq[:],
                                        op=AL.subtract)
                rstd = wrk.tile([64, NPC], F32, tag=f"rstd{tag}")
                nc.scalar.activation(rstd[:], var[:], AF.Sqrt, bias=epsc[:])
                nc.vector.reciprocal(rstd[:], rstd[:])
                for b in range(B):
                    xo = big.tile([64, NPC], F32, tag=f"xo{tag}{h}{b}")
                    nc.vector.tensor_tensor(out=xo[:], in0=raw[(h, b)][:],
                                            in1=mean[:], op=AL.subtract)
                    nc.vector.tensor_tensor(out=xo[:], in0=xo[:], in1=rstd[:],
                                            op=AL.mult)
                    if leaky:
                        lk2 = wrk.tile([64, NPC], F32, tag=f"lk2{tag}")
                        nc.vector.tensor_scalar_mul(lk2[:], xo[:], 0.2)
                        nc.vector.tensor_tensor(out=xo[:], in0=xo[:],
                                                in1=lk2[:], op=AL.max)
                    if tag == "b":
                        nc.gpsimd.dma_start(
                            _sap(dbg1_h, (h * B + b) * NHID * NPC,
                                 [[NPC, NHID], [1, NPC]]), xo[:])
                    out[(h, b)] = xo
            return out

        def rhs_col0(b, h, dims=False):
            if dims:
                return [[48, NST], [24, 2]]
            return 12 * b + h

        def srep0(srep, b, h):
            v = srep[:].rearrange("p (a c) -> p c a", c=2)
            for kp in range(2):
                nc.sync.dma_start(
                    v[:, kp, :],
                    _sap(rs_h, (32 * (b + 2 * kp) + h) * NST,
                         [[0, 64], [1, NST]]))

        x0t = agg_norm(a0out, uT, NH0, 528, 48 * NST, rhs_col0, srep0, True, "a")

        if stop_after <= 3:
            nc.compile()
            return nc
        # ---------------- AllGather of layer-0 outputs
        for h in range(NH0):
            for b in range(B):
                nc.sync.dma_start(
                    _sap(agin_h, h * NHID * RPC + b * NPC, [[RPC, NHID], [1, NPC]]),
                    x0t[(h, b)][:])
        nc.gpsimd.collective_compute(
            "AllGather", AL.bypass, replica_groups=[list(range(N_CORES))],
            ins=[agin_h], outs=[agout_h])

        xg = []
        for h in range(NH0):
            xt = big.tile([64, B * N], F32,
                          tag=(f"uT{h}" if h < 7 else "eadj"), name=f"xg{h}")
            for b in range(B):
                nc.sync.dma_start(
                    xt[:, N * b:N * (b + 1)].rearrange(
                        "p (c n) -> p c n", c=N_CORES),
                    _sap(agout_h, h * NHID * RPC + NPC * b,
                         [[RPC, 64], [NH0 * NHID * RPC, N_CORES], [1, NPC]]))
            xg.append(xt)

        if stop_after <= 4:
            nc.compile()
            return nc
        # layer-1 a1 for own rows -> a1h1_h [RPC, NH1] (rows = 2n+b)
        for b in range(B):
            pa11 = ps_m.tile([NPC, NH1], F32, tag="ps_m")
            for h0 in range(NH0):
                nc.tensor.matmul(pa11[:], x0t[(h0, b)][:], v1k1[:, h0, :],
                                 start=(h0 == 0), stop=False)
            nc.tensor.matmul(pa11[:], ones1[:, 0:NPC], v1kK[:],
                             start=False, stop=True)
            a1o1 = wrk.tile([NPC, NH1], F32, tag="a1o1")
            nc.vector.tensor_copy(a1o1[:], pa11[:])
            nc.sync.dma_start(
                _sap(a1h1_h, b * NH1, [[2 * NH1, NPC], [1, NH1]]), a1o1[:])

        # ---------------- layer-1 A-matmuls
        a1out = []
        for ichk, (g0, sz, b) in enumerate(node_chunks):
            t = big.tile([64, 264], F32, tag=f"a0out{ichk}", name=f"a1out{ichk}")
            p = ps_m.tile([64, 264], F32, tag="ps_m")
            for h0 in range(NH0):
                nc.tensor.matmul(p[0:sz, :], xg[h0][:, g0:g0 + sz],
                                 rhs1[:, h0, :],
                                 start=(h0 == 0), stop=(h0 == NH0 - 1))
            nc.vector.tensor_tensor(out=t[0:sz, :], in0=p[0:sz, :],
                                    in1=brep1[0:sz, :], op=AL.add)
            a1out.append((t, g0, sz, b))

        extract_a2(a1out, NH1, 264, a2h1_h, "1")

        a2rep1 = big.tile([128, N], F32)
        for r in range(32):
            nc.sync.dma_start(a2rep1[4 * r:4 * r + NH1, :], a2h1_h[r % 2])

        # ---------------- phase B attention (4 heads, 32 rows/supertile)
        uT1_tags = ["xfg", "xf1g", "nfg", "nf1g", "xfo", "xf1o", "nfo"]
        uT1 = [big.tile([64, 512], F32, tag=uT1_tags[c], name=f"uT1{c}") for c in range(7)]
        rs1_all = wrk.tile([128, 4], F32, tag="rs1_all")
        for s in range(4):
            nrow = min(32, RPC - 32 * s)
            cp1 = bas.tile([128, N], F32, tag="cp1")
            if nrow < 32:
                nc.vector.memset(cp1[:], 0.0)
            nc.sync.dma_start(
                cp1[0:4 * nrow, :],
                _sap(cp1_h, 32 * s * NH1 * N, [[N, 4 * nrow], [1, N]]))
            cp1v = cp1[:]
            a1c1 = wrk.tile([128, 1], F32, tag="a1c1")
            if nrow < 32:
                nc.vector.memset(a1c1[:], 0.0)
            nc.sync.dma_start(a1c1[0:4 * nrow, :],
                              _sap(a1h1_h, 128 * s, [[1, 4 * nrow], [1, 1]]))
            pair = wrk.tile([128, N], F32, tag="pair")
            nc.vector.tensor_scalar_add(pair[:], a2rep1[:], a1c1[:])
            lk = wrk.tile([128, N], F32, tag="lk")
            nc.vector.tensor_scalar_mul(lk[:], pair[:], 0.2)
            nc.vector.tensor_tensor(out=lk[:], in0=lk[:], in1=pair[:], op=AL.max)
            cc = wrk.tile([128, N], F32, tag="cc")
            nc.vector.tensor_tensor(out=cc[:], in0=lk[:], in1=cp1v, op=AL.add)
            u = wrk.tile([128, N], F32, tag="u")
            nc.scalar.activation(u[:], cc[:], AF.Exp,
                                 accum_out=rs1_all[:, s:s + 1])
            for c in range(7):
                w = min(64, N - 64 * c)
                ptr = ps_t.tile([64, 128], F32, tag="ps_t")
                nc.tensor.transpose(ptr[0:w, :], u[:, 64 * c:64 * c + w],
                                    ident[:])
                nc.vector.tensor_copy(uT1[c][0:w, 128 * s:128 * s + 128], ptr[0:w, :])

        if stop_after <= 5:
            nc.compile()
            return nc
        rcp1 = wrk.tile([128, 4], F32, tag="rcp1")
        nc.vector.reciprocal(rcp1[:], rs1_all[:])
        nc.sync.dma_start(rs1_h, rcp1[:])

        def rhs_col1(b, h, dims=False):
            if dims:
                return [[8, NPC]]
            return 4 * b + h

        def srep1(srep, b, h):
            for sidx in range(4):
                w = 16 if sidx < 3 else 2
                nc.sync.dma_start(
                    srep[:, 16 * sidx:16 * sidx + w],
                    _sap(rs1_h, 16 * b + 4 * h + sidx, [[0, 64], [32, w]]))

        x1t = agg_norm(a1out, uT1, NH1, 264, 512, rhs_col1, srep1, False, "b")

        # ---------------- head mean, relu, output MLP
        for b in range(B):
            acc = wrk.tile([NHID, NPC], F32, tag="hmacc")
            nc.vector.tensor_tensor(out=acc[:], in0=x1t[(0, b)][:],
                                    in1=x1t[(1, b)][:], op=AL.add)
            nc.vector.tensor_tensor(out=acc[:], in0=acc[:], in1=x1t[(2, b)][:],
                                    op=AL.add)
            nc.vector.tensor_tensor(out=acc[:], in0=acc[:], in1=x1t[(3, b)][:],
                                    op=AL.add)
            x2 = wrk.tile([NHID, NPC], F32, tag="x2")
            nc.scalar.activation(x2[:], acc[:], AF.Relu, scale=0.25)
            p1 = ps_m.tile([32, NPC], F32, tag="ps_m")
            nc.tensor.matmul(p1[:], wout[:, 0:32], x2[:], start=True, stop=True)
            y1 = wrk.tile([32, NPC], F32, tag="y1")
            nc.scalar.activation(y1[:], p1[:], AF.Relu, bias=bout[:, 0:1])
            p2 = ps_m.tile([1, NPC], F32, tag="ps_m")
            nc.tensor.matmul(p2[:], wout[0:32, 32:33], y1[:], start=True,
                             stop=True)
            yf = wrk.tile([1, NPC], F32, tag="yf")
            nc.vector.tensor_scalar_add(yf[:], p2[:], bout[0:1, 1:2])
            nc.sync.dma_start(_sap(out_d, b * NPC, [[1, 1], [1, NPC]]), yf[:])

    nc.compile()
    return nc


_CACHE = {}
LAST = [None, None]


# ================================================================== host entry
def kernel(x_c, x_d, x_tw, x_dual, edge, adj, params):
    x_c = np.asarray(x_c, np.float32)
    x_d = np.asarray(x_d, np.float32)
    x_tw = np.asarray(x_tw, np.float32)
    x_dual = np.asarray(x_dual, np.float32)
    edge = np.asarray(edge, np.float32)
    adj = np.asarray(adj, np.float32)

    dv = _derive(params)
    KB = dv['KB']
    key = ('prog', KB)
    if key not in _CACHE:
        _CACHE[key] = build_program(KB)
    nc = _CACHE[key]

    # channel-major raw node features [6, B*N] (global (b, n) order)
    xs = np.concatenate([x_c, x_d, x_tw, x_dual], axis=-1)  # [B, N, 6]
    xall = xs.reshape(B * N, 6).T.copy()                     # [6, 800]

    nbias2 = np.stack([dv['bn'], dv['bemb']], axis=1)
    basv = np.stack([dv['bas_scale'], dv['bas_bias']], axis=1)
    wout = np.zeros((NHID, 33), np.float32)
    wout[:, 0:32] = dv['W1_T']
    wout[0:32, 32] = dv['W2_T'][:, 0]
    bout = np.zeros((32, 2), np.float32)
    bout[:, 0] = dv['b1']
    bout[0, 1] = dv['b2']

    common = {
        "xall": xall, "nconst": dv['Wn_T'], "nbias": nbias2,
        "wemb": dv['Wemb_T'], "basv": basv, "gam": dv['Gamma'],
        "rhs0": dv['rhs0'], "brep0": dv['bias0'].reshape(1, 528),
        "v1k0": dv['v1k0'], "rhs1": dv['rhs1'],
        "brep1": dv['bias1'].reshape(1, 264), "v1k1": dv['v1k1'],
        "wout": wout, "bout": bout,
    }

    in_maps = []
    for c in range(N_CORES):
        nodes = np.arange(NPC * c, NPC * (c + 1))
        # rows ordered (2*n_local + b)
        rows_b = np.repeat([0, 1], 1)  # pattern helper
        eadj = np.zeros((RPC, 2 * N), np.float32)
        xown = np.zeros((6, RPC), np.float32)
        for nl in range(NPC):
            for b in range(B):
                r = 2 * nl + b
                eadj[r, 0:N] = edge[b, nodes[nl]]
                eadj[r, N:2 * N] = adj[b, nodes[nl]]
                xown[:, r] = xall[:, b * N + nodes[nl]]
        m = dict(common)
        m["eadj"] = eadj
        m["xown"] = xown
        in_maps.append(m)

    import time as _time
    _t0 = _time.time()
    res = bass_utils.run_bass_kernel_spmd(nc, in_maps, list(range(N_CORES)))
    LAST[0] = res
    LAST[1] = (_time.time() - _t0) * 1e9

    out = np.zeros((B, N), np.float32)
    for c in range(N_CORES):
        y = res.results[c]["out"][0]          # [RPC] = (b*NPC + n) order
        for b in range(B):
            out[b, NPC * c:NPC * (c + 1)] = y[b * NPC:(b + 1) * NPC]
    return out
